# revision 18
# baseline (speedup 1.0000x reference)
"""Trainium2 Bass kernel for MoE feed-forward (nn_MoEFeedForward_12292196401617).

Reference computation (per batch b of 32, N=1024 tokens, DIM=1024):
    h      = gelu_erf(x @ fc1_w.T + fc1_b)                  # [B,N,HID=4096]
    shared = h @ fc2_w.T + fc2_b                            # [B,N,768]
    expert = h @ experts_w[idx[b]].T + experts_b[idx[b]]    # [B,N,256]
    out    = concat([shared, expert], -1)                   # [B,N,1024]

Strategy: data-parallel over batch across 8 NeuronCores (4 batches/core).
The expert gather is resolved on the host (indices are per-batch), so the
device program is pure dense matmul. Everything is laid out
feature-on-partitions / tokens-on-free-dim, so all host->device tensors are
pre-transposed on the host. Weights/activations are cast to fp16 (the PE
runs fp16 at 1 cycle/row like bf16 — 4x faster than fp32 — but with 10
mantissa bits; end-to-end rel err ~4e-4) and weights stay resident in SBUF;
accumulation is fp32 in PSUM; gelu runs on the scalar engine (erf-exact
Gelu) fused with the fc1 bias add; the fc2/expert bias add is fused into
the PSUM->SBUF eviction. Measured ~907us on hardware, ~98% PE occupancy,
216 ns/matmul = the N=512 issue floor.
"""

import sys

sys.path.insert(0, "/opt/trn_rl_repo")

import numpy as np

B, N, DIM = 32, 1024, 1024
HID = 4096
PART = 256
OUT = 1024
SHARED = OUT - PART  # 768
E = 16

NCORES = 8
BPC = B // NCORES        # batches per core = 4
TOK = BPC * N            # tokens per core  = 4096
TT = 512                 # token tile
NTILES = TOK // TT       # 8  (2 tiles per batch)
P = 128
KC = DIM // P            # 8  fc1 contraction chunks
HC = HID // P            # 32 hidden chunks
OC = OUT // P            # 8  output chunks (6 shared + 2 expert)
SC = SHARED // P         # 6

_CACHE: dict = {}


def _build_program():
    import concourse.tile as tile
    from concourse import bacc, mybir

    bf16 = mybir.dt.float16  # fp16: same PE rate as bf16, 8x the mantissa
    f32 = mybir.dt.float32
    GELU = mybir.ActivationFunctionType.Gelu
    IDENT = mybir.ActivationFunctionType.Identity

    nc = bacc.Bacc()
    xT_d = nc.declare_dram_parameter("xT", [DIM, TOK], bf16, isOutput=False)
    w1T_d = nc.declare_dram_parameter("w1T", [DIM, HID], bf16, isOutput=False)
    b1T_d = nc.declare_dram_parameter("b1T", [P, HC], f32, isOutput=False)
    w2T_d = nc.declare_dram_parameter("w2T", [HID, SHARED], bf16, isOutput=False)
    weT_d = nc.declare_dram_parameter("weT", [BPC, HID, PART], bf16, isOutput=False)
    b2T_d = nc.declare_dram_parameter("b2T", [P, BPC * OC], f32, isOutput=False)
    outT_d = nc.declare_dram_parameter("outT", [OUT, TOK], f32, isOutput=True)

    with tile.TileContext(nc) as tc:
        with (
            tc.tile_pool(name="wsb", bufs=1) as wsb,      # resident weights
            tc.tile_pool(name="wesb", bufs=1) as wesb,    # expert weights (per batch)
            tc.tile_pool(name="bsb", bufs=1) as bsb,      # biases
            tc.tile_pool(name="xsb", bufs=2) as xsb,      # x tiles, double buffered
            tc.tile_pool(name="hsb", bufs=1) as hsb,      # gelu output chunks
            tc.tile_pool(name="osb", bufs=4) as osb,      # out staging
            tc.tile_pool(name="hps", bufs=2, space="PSUM") as hps,
            tc.tile_pool(name="ops", bufs=6, space="PSUM") as ops,
        ):
            # ---- load order matters: the first fc1 chain needs x tile 0 +
            # w1 only; w2 / expert weights aren't read until the PE is ~55us
            # in, so they load behind the critical path.
            b1_t = bsb.tile([P, HC], f32, tag="b1")
            nc.gpsimd.dma_start(b1_t[:], b1T_d[:, :])
            b2_t = bsb.tile([P, BPC * OC], f32, tag="b2")
            nc.gpsimd.dma_start(b2_t[:], b2T_d[:, :])

            # PE warmup: dummy matmuls on an uninitialized scratch tile keep
            # the PE busy from the preamble until the first x/w1 bytes land,
            # so the HAM clock gate starts ramping toward 8/8 immediately.
            # The results are never read.
            scr = bsb.tile([P, TT], bf16, tag="scr")
            nc.vector.memset(scr[:], 0.0)
            for _ in range(30):
                wp = hps.tile([P, TT], f32, tag="hps", name="warm")
                nc.tensor.matmul(
                    wp[:, 0:256], scr[:, 0:P], scr[:, 0:256], start=True, stop=True
                )

            def load_x(ti, engine=None):
                t0 = ti * TT
                tiles = []
                for kc in range(KC):
                    t = xsb.tile([P, TT], bf16, tag=f"x_{kc}")
                    (engine or nc.sync).dma_start(
                        t[:], xT_d[kc * P:(kc + 1) * P, t0:t0 + TT]
                    )
                    tiles.append(t)
                return tiles

            def load_we(b, we_t):
                for hc in range(HC):
                    t = wesb.tile([P, PART], bf16, tag=f"we_{hc}")
                    nc.sync.dma_start(t[:], weT_d[b, hc * P:(hc + 1) * P, :])
                    we_t[hc] = t

            # x tile 0 via SWDGE (gpsimd): software DGE first bytes land at
            # ~3us vs ~9us for the hardware DGE path, and the HWDGE queues
            # are busy with w1 anyway.
            x_pending = load_x(0, engine=nc.gpsimd)

            # w1 loads in 1024-wide column slices (2KB packets halve the
            # per-packet DMA overhead), earliest-needed first: the fc1 chain
            # for hid chunk hc only reads w1 columns [hc*128,(hc+1)*128), so
            # the PE can start once slice 0 (hid chunks 0..7) has landed.
            w1_t = [
                wsb.tile([P, HID], bf16, tag=f"w1_{kc}", name=f"w1_{kc}")
                for kc in range(KC)
            ]
            HS = 1024
            for q in range(HID // HS):
                for kc in range(KC):
                    nc.sync.dma_start(
                        w1_t[kc][:, q * HS:(q + 1) * HS],
                        w1T_d[kc * P:(kc + 1) * P, q * HS:(q + 1) * HS],
                    )

            w2_t = []
            for hc in range(HC):
                t = wsb.tile([P, SHARED], bf16, tag=f"w2_{hc}")
                nc.sync.dma_start(t[:], w2T_d[hc * P:(hc + 1) * P, :])
                w2_t.append(t)

            we_t = [None] * HC  # current batch's expert weight chunks
            load_we(0, we_t)

            for ti in range(NTILES):
                b = ti // (NTILES // BPC)
                t0 = ti * TT
                if ti % (NTILES // BPC) == 0 and ti > 0:
                    load_we(b, we_t)

                x_t = x_pending
                if ti + 1 < NTILES:
                    x_pending = load_x(ti + 1)

                # fc1 + erf-gelu: h^T[hid, tok] per 128-row chunk
                h_t = []
                for hc in range(HC):
                    acc = hps.tile([P, TT], f32, tag="hps")
                    for kc in range(KC):
                        nc.tensor.matmul(
                            acc[:],
                            w1_t[kc][:, hc * P:(hc + 1) * P],
                            x_t[kc][:],
                            start=(kc == 0),
                            stop=(kc == KC - 1),
                        )
                    h = hsb.tile([P, TT], bf16, tag=f"h_{hc}")
                    nc.scalar.activation(
                        h[:], acc[:], GELU, bias=b1_t[:, hc:hc + 1], scale=1.0
                    )
                    h_t.append(h)

                # fc2 (shared) + expert projection: out^T[out, tok]
                for oc in range(OC):
                    acc = ops.tile([P, TT], f32, tag="ops")
                    for hc in range(HC):
                        if oc < SC:
                            w = w2_t[hc][:, oc * P:(oc + 1) * P]
                        else:
                            w = we_t[hc][:, (oc - SC) * P:(oc - SC + 1) * P]
                        nc.tensor.matmul(
                            acc[:], w, h_t[hc][:],
                            start=(hc == 0), stop=(hc == HC - 1),
                        )
                    o = osb.tile([P, TT], f32, tag="o")
                    nc.scalar.activation(
                        o[:], acc[:], IDENT,
                        bias=b2_t[:, b * OC + oc:b * OC + oc + 1], scale=1.0,
                    )
                    nc.sync.dma_start(outT_d[oc * P:(oc + 1) * P, t0:t0 + TT], o[:])

    nc.finalize()
    return nc


def _get_program():
    if "nc" not in _CACHE:
        _CACHE["nc"] = _build_program()
    return _CACHE["nc"]


def _prep_in_maps(x, indices, fc1_w, fc1_b, fc2_w, fc2_b, experts_w, experts_b):
    bf16 = np.float16
    x = np.asarray(x, dtype=np.float32)
    indices = np.asarray(indices).astype(np.int64)
    fc1_w = np.asarray(fc1_w, dtype=np.float32)
    fc1_b = np.asarray(fc1_b, dtype=np.float32)
    fc2_w = np.asarray(fc2_w, dtype=np.float32)
    fc2_b = np.asarray(fc2_b, dtype=np.float32)
    experts_w = np.asarray(experts_w, dtype=np.float32)
    experts_b = np.asarray(experts_b, dtype=np.float32)

    w1T = fc1_w.T.astype(bf16)                       # [DIM, HID]
    b1T = np.ascontiguousarray(fc1_b.reshape(HC, P).T)   # [P, HC]
    w2T = fc2_w.T.astype(bf16)                       # [HID, SHARED]

    in_maps = []
    for c in range(NCORES):
        idx = indices[c * BPC:(c + 1) * BPC]         # [BPC]
        xs = x[c * BPC:(c + 1) * BPC]                # [BPC, N, DIM]
        xT = xs.reshape(TOK, DIM).T.astype(bf16)     # [DIM, TOK]
        weT = experts_w[idx].transpose(0, 2, 1).astype(bf16)  # [BPC, HID, PART]
        b2 = np.concatenate(
            [np.broadcast_to(fc2_b, (BPC, SHARED)), experts_b[idx]], axis=1
        )                                            # [BPC, OUT]
        b2T = np.ascontiguousarray(
            b2.reshape(BPC, OC, P).transpose(2, 0, 1).reshape(P, BPC * OC)
        ).astype(np.float32)                         # [P, BPC*OC]
        in_maps.append({
            "xT": xT, "w1T": w1T, "b1T": b1T, "w2T": w2T,
            "weT": weT, "b2T": b2T,
        })
    return in_maps


def _assemble_output(results):
    out = np.empty((B, N, OUT), dtype=np.float32)
    for c in range(NCORES):
        outT = results[c]["outT"]                    # [OUT, TOK]
        out[c * BPC:(c + 1) * BPC] = outT.T.reshape(BPC, N, OUT)
    return out


def run_on_device(inputs: dict, trace: bool = False):
    """Run the SPMD program; returns (full_output, BassKernelResults)."""
    from concourse.bass_utils import run_bass_kernel_spmd

    nc = _get_program()
    in_maps = _prep_in_maps(**inputs)
    res = run_bass_kernel_spmd(nc, in_maps, list(range(NCORES)), trace=trace)
    return _assemble_output(res.results), res


def kernel(**inputs) -> np.ndarray:
    out, _ = run_on_device(inputs, trace=False)
    return out


# revision 24
# speedup vs baseline: 1.0027x; 1.0027x over previous
"""Trainium2 Bass kernel for MoE feed-forward (nn_MoEFeedForward_12292196401617).

Reference computation (per batch b of 32, N=1024 tokens, DIM=1024):
    h      = gelu_erf(x @ fc1_w.T + fc1_b)                  # [B,N,HID=4096]
    shared = h @ fc2_w.T + fc2_b                            # [B,N,768]
    expert = h @ experts_w[idx[b]].T + experts_b[idx[b]]    # [B,N,256]
    out    = concat([shared, expert], -1)                   # [B,N,1024]

Strategy: data-parallel over batch across 8 NeuronCores (4 batches/core).
The expert gather is resolved on the host (indices are per-batch), so the
device program is pure dense matmul. Everything is laid out
feature-on-partitions / tokens-on-free-dim, and every device input is
host-packed so that each DMA writes one SBUF tile whose per-partition row
is a single multi-KB contiguous DRAM run (DMA engines are packet-rate
limited; 8KB packets instead of 1KB quadruple early bandwidth).
Weights/activations are fp16 (PE runs fp16 at 1 cycle/row like bf16 — 4x
faster than fp32 — with 10 mantissa bits; end-to-end rel err ~4e-4),
weights stay resident in SBUF; accumulation is fp32 in PSUM; the erf-exact
Gelu on the scalar engine applies the fc1 bias during PSUM eviction and an
Identity activation applies the fc2/expert bias. Measured ~900us on
hardware, ~98% PE occupancy, 216 ns/matmul = the N=512 issue floor.
"""

import sys

sys.path.insert(0, "/opt/trn_rl_repo")

import numpy as np

B, N, DIM = 32, 1024, 1024
HID = 4096
PART = 256
OUT = 1024
SHARED = OUT - PART  # 768
E = 16

NCORES = 8
BPC = B // NCORES        # batches per core = 4
TOK = BPC * N            # tokens per core  = 4096
TT = 512                 # token tile
NTILES = TOK // TT       # 8  (2 tiles per batch)
P = 128
KC = DIM // P            # 8  fc1 contraction chunks
HC = HID // P            # 32 hidden chunks
OC = OUT // P            # 8  output chunks (6 shared + 2 expert)
SC = SHARED // P         # 6
W1Q = 16                 # w1 column-slice groups (HC/W1Q = 2 hid chunks each)
W2G = 4                  # w2 hid chunks packed per DMA

_CACHE: dict = {}


def _build_program():
    import concourse.tile as tile
    from concourse import bacc, mybir

    fp16 = mybir.dt.float16
    f32 = mybir.dt.float32
    GELU = mybir.ActivationFunctionType.Gelu
    IDENT = mybir.ActivationFunctionType.Identity

    HQ = HID // W1Q          # 1024 hid cols per w1 slice group
    nc = bacc.Bacc()
    # packed layouts: [.., P, ..] second-to-last dim is the SBUF partition,
    # the trailing dims are one contiguous row per partition.
    xP_d = nc.declare_dram_parameter("xP", [NTILES, P, KC * TT], fp16, isOutput=False)
    w1P_d = nc.declare_dram_parameter("w1P", [W1Q, P, KC * HQ], fp16, isOutput=False)
    b1T_d = nc.declare_dram_parameter("b1T", [P, HC], f32, isOutput=False)
    w2P_d = nc.declare_dram_parameter("w2P", [HC // W2G, P, W2G * SHARED], fp16, isOutput=False)
    weP_d = nc.declare_dram_parameter("weP", [BPC, P, HC * PART], fp16, isOutput=False)
    b2T_d = nc.declare_dram_parameter("b2T", [P, BPC * OC], f32, isOutput=False)
    outT_d = nc.declare_dram_parameter("outT", [OUT, TOK], f32, isOutput=True)

    with tile.TileContext(nc) as tc:
        with (
            tc.tile_pool(name="wsb", bufs=1) as wsb,      # resident weights
            tc.tile_pool(name="wesb", bufs=1) as wesb,    # expert weights (per batch)
            tc.tile_pool(name="bsb", bufs=1) as bsb,      # biases
            tc.tile_pool(name="xsb", bufs=2) as xsb,      # x tiles, double buffered
            tc.tile_pool(name="hsb", bufs=1) as hsb,      # gelu output chunks
            tc.tile_pool(name="osb", bufs=4) as osb,      # out staging
            tc.tile_pool(name="hps", bufs=2, space="PSUM") as hps,
            tc.tile_pool(name="ops", bufs=6, space="PSUM") as ops,
        ):
            # ---- load order matters: the first fc1 chain needs x tile 0 +
            # w1 slice 0 only; w2 / expert weights aren't read until the PE
            # is ~55us in, so they load behind the critical path.
            b1_t = bsb.tile([P, HC], f32, tag="b1")
            nc.gpsimd.dma_start(b1_t[:], b1T_d[:, :])
            b2_t = bsb.tile([P, BPC * OC], f32, tag="b2")
            nc.gpsimd.dma_start(b2_t[:], b2T_d[:, :])

            # PE warmup: dummy matmuls on a memset scratch tile keep the PE
            # busy from the preamble until the first x/w1 bytes land, so the
            # HAM clock gate reaches 8/8 before real work starts. The
            # results are never read.
            scr = bsb.tile([P, TT], fp16, tag="scr")
            nc.vector.memset(scr[:], 0.0)
            for _ in range(24):
                wp = hps.tile([P, TT], f32, tag="hps", name="warm")
                nc.tensor.matmul(
                    wp[:, 0:256], scr[:, 0:P], scr[:, 0:256], start=True, stop=True
                )

            def load_x(ti):
                # two DMAs per token tile (4KB rows each): the fc1 chains'
                # kc=0..3 matmuls only depend on the first half, so the PE
                # can start before the full tile lands.
                t = xsb.tile([P, KC * TT], fp16, tag="xt", name="xt")
                half = KC * TT // 2
                nc.sync.dma_start(t[:, 0:half], xP_d[ti, :, 0:half])
                nc.sync.dma_start(t[:, half:], xP_d[ti, :, half:])
                return t

            def load_we(b):
                # one DMA per batch: [P, HC*PART] with 16KB rows
                t = wesb.tile([P, HC * PART], fp16, tag="we", name="we")
                nc.sync.dma_start(t[:], weP_d[b])
                return t

            x_pend = load_x(0)

            # w1: W1Q slice-group tiles [P, KC*HQ], one DMA each (8KB rows),
            # earliest-needed first. Chain hc uses group hc // (HC//W1Q).
            w1_t = []
            for q in range(W1Q):
                t = wsb.tile([P, KC * HQ], fp16, tag=f"w1_{q}", name=f"w1_{q}")
                nc.sync.dma_start(t[:], w1P_d[q])
                w1_t.append(t)

            # w2: HC//W2G group tiles [P, W2G*SHARED], one DMA each (6KB rows)
            w2_t = []
            for g in range(HC // W2G):
                t = wsb.tile([P, W2G * SHARED], fp16, tag=f"w2_{g}", name=f"w2_{g}")
                nc.sync.dma_start(t[:], w2P_d[g])
                w2_t.append(t)

            we_cur = load_we(0)

            HPG = HC // W1Q  # hid chunks per w1 slice group = 8
            for ti in range(NTILES):
                b = ti // (NTILES // BPC)
                t0 = ti * TT
                if ti % (NTILES // BPC) == 0 and ti > 0:
                    we_cur = load_we(b)

                x_t = x_pend
                if ti + 1 < NTILES:
                    x_pend = load_x(ti + 1)

                # fc1 + erf-gelu: h^T[hid, tok] per 128-row chunk
                h_t = []
                for hc in range(HC):
                    q, r = divmod(hc, HPG)
                    acc = hps.tile([P, TT], f32, tag="hps")
                    for kc in range(KC):
                        nc.tensor.matmul(
                            acc[:],
                            w1_t[q][:, kc * HQ + r * P:kc * HQ + r * P + P],
                            x_t[:, kc * TT:(kc + 1) * TT],
                            start=(kc == 0),
                            stop=(kc == KC - 1),
                        )
                    h = hsb.tile([P, TT], fp16, tag=f"h_{hc}")
                    nc.scalar.activation(
                        h[:], acc[:], GELU, bias=b1_t[:, hc:hc + 1], scale=1.0
                    )
                    h_t.append(h)

                # fc2 (shared) + expert projection: out^T[out, tok]
                for oc in range(OC):
                    acc = ops.tile([P, TT], f32, tag="ops")
                    for hc in range(HC):
                        if oc < SC:
                            g, j = divmod(hc, W2G)
                            w = w2_t[g][:, j * SHARED + oc * P:j * SHARED + (oc + 1) * P]
                        else:
                            w = we_cur[:, hc * PART + (oc - SC) * P:hc * PART + (oc - SC + 1) * P]
                        nc.tensor.matmul(
                            acc[:], w, h_t[hc][:],
                            start=(hc == 0), stop=(hc == HC - 1),
                        )
                    o = osb.tile([P, TT], f32, tag="o")
                    nc.scalar.activation(
                        o[:], acc[:], IDENT,
                        bias=b2_t[:, b * OC + oc:b * OC + oc + 1], scale=1.0,
                    )
                    nc.sync.dma_start(outT_d[oc * P:(oc + 1) * P, t0:t0 + TT], o[:])

    nc.finalize()
    return nc


def _get_program():
    if "nc" not in _CACHE:
        _CACHE["nc"] = _build_program()
    return _CACHE["nc"]


def _prep_in_maps(x, indices, fc1_w, fc1_b, fc2_w, fc2_b, experts_w, experts_b):
    fp16 = np.float16
    x = np.asarray(x, dtype=np.float32)
    indices = np.asarray(indices).astype(np.int64)
    fc1_w = np.asarray(fc1_w, dtype=np.float32)
    fc1_b = np.asarray(fc1_b, dtype=np.float32)
    fc2_w = np.asarray(fc2_w, dtype=np.float32)
    fc2_b = np.asarray(fc2_b, dtype=np.float32)
    experts_w = np.asarray(experts_w, dtype=np.float32)
    experts_b = np.asarray(experts_b, dtype=np.float32)

    HQ = HID // W1Q
    # w1P[q, p, kc, c] = fc1_w.T[kc*P+p, q*HQ+c] ; rows are KC*HQ fp16 = 8KB
    w1T = fc1_w.T                                         # [DIM, HID]
    w1P = np.ascontiguousarray(
        w1T.reshape(KC, P, W1Q, HQ).transpose(2, 1, 0, 3)
    ).astype(fp16).reshape(W1Q, P, KC * HQ)
    b1T = np.ascontiguousarray(fc1_b.reshape(HC, P).T)    # [P, HC]
    # w2P[g, p, j, s] = fc2_w.T[(g*W2G+j)*P+p, s]
    w2P = np.ascontiguousarray(
        fc2_w.T.reshape(HC // W2G, W2G, P, SHARED).transpose(0, 2, 1, 3)
    ).astype(fp16).reshape(HC // W2G, P, W2G * SHARED)

    in_maps = []
    for c in range(NCORES):
        idx = indices[c * BPC:(c + 1) * BPC]              # [BPC]
        xs = x[c * BPC:(c + 1) * BPC]                     # [BPC, N, DIM]
        xT = xs.reshape(TOK, DIM).T                       # [DIM, TOK]
        # xP[ti, p, kc, t] = xT[kc*P+p, ti*TT+t] ; rows are KC*TT fp16 = 8KB
        xP = np.ascontiguousarray(
            xT.reshape(KC, P, NTILES, TT).transpose(2, 1, 0, 3)
        ).astype(fp16).reshape(NTILES, P, KC * TT)
        # weP[b, p, hc, s] = experts_w[idx[b]].T[hc*P+p, s] ; rows 16KB
        weT = experts_w[idx].transpose(0, 2, 1)           # [BPC, HID, PART]
        weP = np.ascontiguousarray(
            weT.reshape(BPC, HC, P, PART).transpose(0, 2, 1, 3)
        ).astype(fp16).reshape(BPC, P, HC * PART)
        b2 = np.concatenate(
            [np.broadcast_to(fc2_b, (BPC, SHARED)), experts_b[idx]], axis=1
        )                                                 # [BPC, OUT]
        b2T = np.ascontiguousarray(
            b2.reshape(BPC, OC, P).transpose(2, 0, 1).reshape(P, BPC * OC)
        ).astype(np.float32)                              # [P, BPC*OC]
        in_maps.append({
            "xP": xP, "w1P": w1P, "b1T": b1T, "w2P": w2P,
            "weP": weP, "b2T": b2T,
        })
    return in_maps


def _assemble_output(results):
    out = np.empty((B, N, OUT), dtype=np.float32)
    for c in range(NCORES):
        outT = results[c]["outT"]                         # [OUT, TOK]
        out[c * BPC:(c + 1) * BPC] = outT.T.reshape(BPC, N, OUT)
    return out


def run_on_device(inputs: dict, trace: bool = False):
    """Run the SPMD program; returns (full_output, BassKernelResults)."""
    from concourse.bass_utils import run_bass_kernel_spmd

    nc = _get_program()
    in_maps = _prep_in_maps(**inputs)
    res = run_bass_kernel_spmd(nc, in_maps, list(range(NCORES)), trace=trace)
    return _assemble_output(res.results), res


def kernel(**inputs) -> np.ndarray:
    out, _ = run_on_device(inputs, trace=False)
    return out


# revision 25
# speedup vs baseline: 1.0029x; 1.0002x over previous
"""Trainium2 Bass kernel for MoE feed-forward (nn_MoEFeedForward_12292196401617).

Reference computation (per batch b of 32, N=1024 tokens, DIM=1024):
    h      = gelu_erf(x @ fc1_w.T + fc1_b)                  # [B,N,HID=4096]
    shared = h @ fc2_w.T + fc2_b                            # [B,N,768]
    expert = h @ experts_w[idx[b]].T + experts_b[idx[b]]    # [B,N,256]
    out    = concat([shared, expert], -1)                   # [B,N,1024]

Strategy: data-parallel over batch across 8 NeuronCores (4 batches/core).
The expert gather is resolved on the host (indices are per-batch), so the
device program is pure dense matmul. Everything is laid out
feature-on-partitions / tokens-on-free-dim, and every device input is
host-packed so that each DMA writes one SBUF tile whose per-partition row
is a single multi-KB contiguous DRAM run (DMA engines are packet-rate
limited; 8KB packets instead of 1KB quadruple early bandwidth).
Weights/activations are fp16 (PE runs fp16 at 1 cycle/row like bf16 — 4x
faster than fp32 — with 10 mantissa bits; end-to-end rel err ~4e-4),
weights stay resident in SBUF; accumulation is fp32 in PSUM; the erf-exact
Gelu on the scalar engine applies the fc1 bias during PSUM eviction and an
Identity activation applies the fc2/expert bias. Measured ~900us on
hardware, ~98% PE occupancy, 216 ns/matmul = the N=512 issue floor.
"""

import sys

sys.path.insert(0, "/opt/trn_rl_repo")

import numpy as np

B, N, DIM = 32, 1024, 1024
HID = 4096
PART = 256
OUT = 1024
SHARED = OUT - PART  # 768
E = 16

NCORES = 8
BPC = B // NCORES        # batches per core = 4
TOK = BPC * N            # tokens per core  = 4096
TT = 512                 # token tile
NTILES = TOK // TT       # 8  (2 tiles per batch)
P = 128
KC = DIM // P            # 8  fc1 contraction chunks
HC = HID // P            # 32 hidden chunks
OC = OUT // P            # 8  output chunks (6 shared + 2 expert)
SC = SHARED // P         # 6
W1Q = 16                 # w1 column-slice groups (HC/W1Q = 2 hid chunks each)
W2G = 4                  # w2 hid chunks packed per DMA

_CACHE: dict = {}


def _build_program():
    import concourse.tile as tile
    from concourse import bacc, mybir

    fp16 = mybir.dt.float16
    f32 = mybir.dt.float32
    GELU = mybir.ActivationFunctionType.Gelu
    IDENT = mybir.ActivationFunctionType.Identity

    HQ = HID // W1Q          # 1024 hid cols per w1 slice group
    nc = bacc.Bacc()
    # packed layouts: [.., P, ..] second-to-last dim is the SBUF partition,
    # the trailing dims are one contiguous row per partition.
    xP_d = nc.declare_dram_parameter("xP", [NTILES, P, KC * TT], fp16, isOutput=False)
    w1P_d = nc.declare_dram_parameter("w1P", [W1Q, P, KC * HQ], fp16, isOutput=False)
    b1T_d = nc.declare_dram_parameter("b1T", [P, HC], f32, isOutput=False)
    w2P_d = nc.declare_dram_parameter("w2P", [HC // W2G, P, W2G * SHARED], fp16, isOutput=False)
    weP_d = nc.declare_dram_parameter("weP", [BPC, P, HC * PART], fp16, isOutput=False)
    b2T_d = nc.declare_dram_parameter("b2T", [P, BPC * OC], f32, isOutput=False)
    outT_d = nc.declare_dram_parameter("outT", [OUT, TOK], f32, isOutput=True)

    with tile.TileContext(nc) as tc:
        with (
            tc.tile_pool(name="wsb", bufs=1) as wsb,      # resident weights
            tc.tile_pool(name="wesb", bufs=1) as wesb,    # expert weights (per batch)
            tc.tile_pool(name="bsb", bufs=1) as bsb,      # biases
            tc.tile_pool(name="xsb", bufs=2) as xsb,      # x tiles, double buffered
            tc.tile_pool(name="hsb", bufs=1) as hsb,      # gelu output chunks
            tc.tile_pool(name="osb", bufs=4) as osb,      # out staging
            tc.tile_pool(name="hps", bufs=2, space="PSUM") as hps,
            tc.tile_pool(name="ops", bufs=6, space="PSUM") as ops,
        ):
            # ---- load order matters: the first fc1 chain needs x tile 0 +
            # w1 slice 0 only; w2 / expert weights aren't read until the PE
            # is ~55us in, so they load behind the critical path.
            b1_t = bsb.tile([P, HC], f32, tag="b1")
            nc.gpsimd.dma_start(b1_t[:], b1T_d[:, :])
            b2_t = bsb.tile([P, BPC * OC], f32, tag="b2")
            nc.gpsimd.dma_start(b2_t[:], b2T_d[:, :])

            # PE warmup: dummy matmuls on a memset scratch tile keep the PE
            # busy from the preamble until the first x/w1 bytes land, so the
            # HAM clock gate reaches 8/8 before real work starts. The
            # results are never read.
            scr = bsb.tile([P, TT], fp16, tag="scr")
            nc.vector.memset(scr[:], 0.0)
            for _ in range(64):
                wp = hps.tile([P, TT], f32, tag="hps", name="warm")
                nc.tensor.matmul(
                    wp[:, 0:256], scr[:, 0:P], scr[:, 0:256], start=True, stop=True
                )

            def load_x(ti):
                # two DMAs per token tile (4KB rows each): the fc1 chains'
                # kc=0..3 matmuls only depend on the first half, so the PE
                # can start before the full tile lands.
                t = xsb.tile([P, KC * TT], fp16, tag="xt", name="xt")
                half = KC * TT // 2
                nc.sync.dma_start(t[:, 0:half], xP_d[ti, :, 0:half])
                nc.sync.dma_start(t[:, half:], xP_d[ti, :, half:])
                return t

            def load_we(b):
                # one DMA per batch: [P, HC*PART] with 16KB rows
                t = wesb.tile([P, HC * PART], fp16, tag="we", name="we")
                nc.sync.dma_start(t[:], weP_d[b])
                return t

            x_pend = load_x(0)

            # w1: W1Q slice-group tiles [P, KC*HQ], one DMA each (8KB rows),
            # earliest-needed first. Chain hc uses group hc // (HC//W1Q).
            w1_t = []
            for q in range(W1Q):
                t = wsb.tile([P, KC * HQ], fp16, tag=f"w1_{q}", name=f"w1_{q}")
                nc.sync.dma_start(t[:], w1P_d[q])
                w1_t.append(t)

            # w2: HC//W2G group tiles [P, W2G*SHARED], one DMA each (6KB rows)
            w2_t = []
            for g in range(HC // W2G):
                t = wsb.tile([P, W2G * SHARED], fp16, tag=f"w2_{g}", name=f"w2_{g}")
                nc.sync.dma_start(t[:], w2P_d[g])
                w2_t.append(t)

            we_cur = load_we(0)

            HPG = HC // W1Q  # hid chunks per w1 slice group = 8
            for ti in range(NTILES):
                b = ti // (NTILES // BPC)
                t0 = ti * TT
                if ti % (NTILES // BPC) == 0 and ti > 0:
                    we_cur = load_we(b)

                x_t = x_pend
                if ti + 1 < NTILES:
                    x_pend = load_x(ti + 1)

                # fc1 + erf-gelu: h^T[hid, tok] per 128-row chunk
                h_t = []
                for hc in range(HC):
                    q, r = divmod(hc, HPG)
                    acc = hps.tile([P, TT], f32, tag="hps")
                    for kc in range(KC):
                        nc.tensor.matmul(
                            acc[:],
                            w1_t[q][:, kc * HQ + r * P:kc * HQ + r * P + P],
                            x_t[:, kc * TT:(kc + 1) * TT],
                            start=(kc == 0),
                            stop=(kc == KC - 1),
                        )
                    h = hsb.tile([P, TT], fp16, tag=f"h_{hc}")
                    nc.scalar.activation(
                        h[:], acc[:], GELU, bias=b1_t[:, hc:hc + 1], scale=1.0
                    )
                    h_t.append(h)

                # fc2 (shared) + expert projection: out^T[out, tok]
                for oc in range(OC):
                    acc = ops.tile([P, TT], f32, tag="ops")
                    for hc in range(HC):
                        if oc < SC:
                            g, j = divmod(hc, W2G)
                            w = w2_t[g][:, j * SHARED + oc * P:j * SHARED + (oc + 1) * P]
                        else:
                            w = we_cur[:, hc * PART + (oc - SC) * P:hc * PART + (oc - SC + 1) * P]
                        nc.tensor.matmul(
                            acc[:], w, h_t[hc][:],
                            start=(hc == 0), stop=(hc == HC - 1),
                        )
                    o = osb.tile([P, TT], f32, tag="o")
                    nc.scalar.activation(
                        o[:], acc[:], IDENT,
                        bias=b2_t[:, b * OC + oc:b * OC + oc + 1], scale=1.0,
                    )
                    nc.sync.dma_start(outT_d[oc * P:(oc + 1) * P, t0:t0 + TT], o[:])

    nc.finalize()
    return nc


def _get_program():
    if "nc" not in _CACHE:
        _CACHE["nc"] = _build_program()
    return _CACHE["nc"]


def _prep_in_maps(x, indices, fc1_w, fc1_b, fc2_w, fc2_b, experts_w, experts_b):
    fp16 = np.float16
    x = np.asarray(x, dtype=np.float32)
    indices = np.asarray(indices).astype(np.int64)
    fc1_w = np.asarray(fc1_w, dtype=np.float32)
    fc1_b = np.asarray(fc1_b, dtype=np.float32)
    fc2_w = np.asarray(fc2_w, dtype=np.float32)
    fc2_b = np.asarray(fc2_b, dtype=np.float32)
    experts_w = np.asarray(experts_w, dtype=np.float32)
    experts_b = np.asarray(experts_b, dtype=np.float32)

    HQ = HID // W1Q
    # w1P[q, p, kc, c] = fc1_w.T[kc*P+p, q*HQ+c] ; rows are KC*HQ fp16 = 8KB
    w1T = fc1_w.T                                         # [DIM, HID]
    w1P = np.ascontiguousarray(
        w1T.reshape(KC, P, W1Q, HQ).transpose(2, 1, 0, 3)
    ).astype(fp16).reshape(W1Q, P, KC * HQ)
    b1T = np.ascontiguousarray(fc1_b.reshape(HC, P).T)    # [P, HC]
    # w2P[g, p, j, s] = fc2_w.T[(g*W2G+j)*P+p, s]
    w2P = np.ascontiguousarray(
        fc2_w.T.reshape(HC // W2G, W2G, P, SHARED).transpose(0, 2, 1, 3)
    ).astype(fp16).reshape(HC // W2G, P, W2G * SHARED)

    in_maps = []
    for c in range(NCORES):
        idx = indices[c * BPC:(c + 1) * BPC]              # [BPC]
        xs = x[c * BPC:(c + 1) * BPC]                     # [BPC, N, DIM]
        xT = xs.reshape(TOK, DIM).T                       # [DIM, TOK]
        # xP[ti, p, kc, t] = xT[kc*P+p, ti*TT+t] ; rows are KC*TT fp16 = 8KB
        xP = np.ascontiguousarray(
            xT.reshape(KC, P, NTILES, TT).transpose(2, 1, 0, 3)
        ).astype(fp16).reshape(NTILES, P, KC * TT)
        # weP[b, p, hc, s] = experts_w[idx[b]].T[hc*P+p, s] ; rows 16KB
        weT = experts_w[idx].transpose(0, 2, 1)           # [BPC, HID, PART]
        weP = np.ascontiguousarray(
            weT.reshape(BPC, HC, P, PART).transpose(0, 2, 1, 3)
        ).astype(fp16).reshape(BPC, P, HC * PART)
        b2 = np.concatenate(
            [np.broadcast_to(fc2_b, (BPC, SHARED)), experts_b[idx]], axis=1
        )                                                 # [BPC, OUT]
        b2T = np.ascontiguousarray(
            b2.reshape(BPC, OC, P).transpose(2, 0, 1).reshape(P, BPC * OC)
        ).astype(np.float32)                              # [P, BPC*OC]
        in_maps.append({
            "xP": xP, "w1P": w1P, "b1T": b1T, "w2P": w2P,
            "weP": weP, "b2T": b2T,
        })
    return in_maps


def _assemble_output(results):
    out = np.empty((B, N, OUT), dtype=np.float32)
    for c in range(NCORES):
        outT = results[c]["outT"]                         # [OUT, TOK]
        out[c * BPC:(c + 1) * BPC] = outT.T.reshape(BPC, N, OUT)
    return out


def run_on_device(inputs: dict, trace: bool = False):
    """Run the SPMD program; returns (full_output, BassKernelResults)."""
    from concourse.bass_utils import run_bass_kernel_spmd

    nc = _get_program()
    in_maps = _prep_in_maps(**inputs)
    res = run_bass_kernel_spmd(nc, in_maps, list(range(NCORES)), trace=trace)
    return _assemble_output(res.results), res


def kernel(**inputs) -> np.ndarray:
    out, _ = run_on_device(inputs, trace=False)
    return out
